# revision 14
# baseline (speedup 1.0000x reference)
"""Fused multi-head attention (2 heads, RoPE-across-heads) on 8 Trainium2 NeuronCores.

Reference computation (per batch b of 4, seq 2048, dim 2048):
    qkv = x @ wqkv; rope mixes the two heads; scores = q'k'^T/32; softmax;
    out = (attn @ v) @ wout + bout

Sharding: core c owns (batch = c//2, seq-half = c%2) -> 1024 query rows.
Each core projects q/k/v for its own 1024 rows, ropes q/k, AllGathers
k'/v within the (2c, 2c+1) pair (chunked, overlapped with the projections),
runs attention for its rows against the full 2048-seq k'/v, and applies the
output projection for its rows.

On-device layouts (partition dim first):
    xT    [dim, rows]      - rhs/stationary for projections
    q'T   [2048, rows]     - head-dim on partitions (chunked [128,16,1024])
    k'T   [2048, rows]     - paired rows: head0-chunk c at 2c*128, head1 at
                             (2c+1)*128, so each AllGather half is contiguous
    v     [4, rows, 512]   - block-column layout so AG halves are contiguous
    P^T   [seq_j, rows]    - exp(scores^T), bf16
    aoT   [2048, rows]     - unnormalized attn-out^T, normalized on write

Softmax skips max-subtraction: scores = q'.k'/32 ~ N(0,1), |scores| < ~8,
so exp is safe in f32 (verified against the reference distribution).
"""

import os
import sys

import numpy as np

if "/opt/trn_rl_repo" not in sys.path:
    sys.path.insert(0, "/opt/trn_rl_repo")

import ml_dtypes

# ---------------------------------------------------------------- constants
B, S, D = 4, 2048, 2048          # batch, seq, model dim
H, HD = 2, 1024                  # heads, head dim
R = 1024                         # query rows per core
N_CORES = 8
SCALE = 1.0 / 32.0               # HD ** -0.5

_NC_CACHE = {}
LAST_RESULT = {}


def _build():
    import concourse.bass as bass
    import concourse.tile as tile
    from concourse import bacc, mybir

    F32 = mybir.dt.float32
    F32R = mybir.dt.float32r
    F16 = mybir.dt.float16
    BF = mybir.dt.bfloat16
    Exp = mybir.ActivationFunctionType.Exp

    NRB = R // 512                             # 512-row blocks

    nc = bacc.Bacc("TRN2", target_bir_lowering=False, debug=False,
                   num_devices=N_CORES)

    xT = nc.dram_tensor("xT", [D, R], BF, kind="ExternalInput").ap()
    wqkv = nc.dram_tensor("wqkv", [D, 3 * D], BF, kind="ExternalInput").ap()
    wout = nc.dram_tensor("wout", [D, D], BF, kind="ExternalInput").ap()
    cost = nc.dram_tensor("cost", [512, R], F16, kind="ExternalInput").ap()
    sint = nc.dram_tensor("sint", [512, R], F16, kind="ExternalInput").ap()
    bias = nc.dram_tensor("bias", [1, D], F32, kind="ExternalInput").ap()
    out = nc.dram_tensor("out", [R, D], BF, kind="ExternalOutput").ap()

    wq_r = wqkv.rearrange("(c p) m -> p c m", p=128)    # [128, 16, 6144]
    x_r = xT.rearrange("(c p) r -> p c r", p=128)       # [128, 16, R]
    c_r = cost.rearrange("(c p) r -> p c r", p=128)     # [128, 4, R]
    s_r = sint.rearrange("(c p) r -> p c r", p=128)

    def bcast_ap(src_ap, nparts, width):
        return bass.AP(tensor=src_ap.tensor, offset=src_ap.offset,
                       ap=[[0, nparts], [1, width]])

    with tile.TileContext(nc) as tc:
        with (
            tc.tile_pool(name="persist", bufs=1) as persist,
            tc.tile_pool(name="psum", bufs=6, space="PSUM") as psp,
            tc.tile_pool(name="dram", bufs=1, space="DRAM") as dram,
        ):
            # ------------------------------------------- persistent buffers
            qT_sb = persist.tile([128, 16, R], BF, tag="qT")
            aoT_sb = persist.tile([128, 16, R], BF, tag="aoT")
            bias_sb = persist.tile([128, D], F32, tag="bias")
            ones_sb = persist.tile([128, 1], BF, tag="ones")
            onesr_sb = persist.tile([1, 128], F32, tag="onesr")
            nc.gpsimd.dma_start(out=bias_sb, in_=bcast_ap(bias, 128, D))
            nc.vector.memset(ones_sb, 1.0)
            nc.vector.memset(onesr_sb, 1.0)

            # DRAM scratch.  k_in rows pair the heads (head0-chunk c at
            # 2c*128, head1-chunk c at (2c+1)*128) so each AllGather half
            # is a contiguous row range.  v_in is block-column: block vc
            # holds cols [vc*512,(vc+1)*512) for all R rows, contiguous.
            # Gather buffers are CHUNK-major so each chunked AllGather's
            # output region is contiguous: k_g = [2 chunks][2 sh][1024, R],
            # v_g = [2 chunks][2 sh][2R, 512].
            k_in = dram.tile([D, R], BF, tag="k_in")
            v_in = dram.tile([4 * R, 512], BF, tag="v_in")
            k_g = dram.tile([2 * D, R], BF, tag="k_g")
            v_g = dram.tile([8 * R, 512], BF, tag="v_g")

            def ag_pair(in_ap, out_ap):
                nc.gpsimd.collective_compute(
                    "AllGather", bass.mybir.AluOpType.bypass,
                    replica_groups=[[0, 1], [2, 3], [4, 5], [6, 7]],
                    ins=[in_ap], outs=[out_ap])

            def k_ag_chunk(half):
                ag_pair(k_in[half * 1024:(half + 1) * 1024, :],
                        k_g[half * 2048:(half + 1) * 2048, :])

            def v_ag_chunk(vc):
                ag_pair(v_in[vc * R:(vc + 1) * R, :],
                        v_g[vc * 2 * R:(vc + 1) * 2 * R, :])

            # =================================================== projections
            with (
                tc.tile_pool(name="proj", bufs=1) as proj,
                tc.tile_pool(name="projs", bufs=1) as projs,
            ):
                # first stationary weights for the k projection, then x, then
                # rope tables (not needed until the first psum chains finish)
                w_first = []
                for c in (0, 8):
                    wt = projs.tile([128, 16, 128], BF, tag="wst", bufs=8)
                    for hh in (0, 8):
                        nc.sync.dma_start(
                            out=wt[:, hh:hh + 8, :],
                            in_=wq_r[:, hh:hh + 8, D + c * 128:D + (c + 1) * 128])
                    w_first.append(wt)
                x_sb = proj.tile([128, 16, R], BF, tag="x")
                for kc in range(16):
                    eng = nc.sync if kc % 2 == 0 else nc.gpsimd
                    eng.dma_start(out=x_sb[:, kc, :], in_=x_r[:, kc, :])
                # hoist the first v-projection weight block; it is consumed
                # only after the whole k projection, so it always prefetches
                wv0 = projs.tile([128, 16, 512], BF, tag="wv", bufs=2)
                for kc in range(0, 16, 4):
                    nc.gpsimd.dma_start(
                        out=wv0[:, kc:kc + 4, :],
                        in_=wq_r[:, kc:kc + 4, 2 * D:2 * D + 512])
                cos_sb = proj.tile([128, 4, R], F16, tag="cos")
                sin_sb = proj.tile([128, 4, R], F16, tag="sin")
                nc.scalar.dma_start(out=cos_sb, in_=c_r)
                nc.scalar.dma_start(out=sin_sb, in_=s_r)

                def load_wst(col0, cc0, dma_eng):
                    wt = projs.tile([128, 16, 128], BF, tag="wst", bufs=8)
                    for hh in (0, 8):
                        dma_eng.dma_start(
                            out=wt[:, hh:hh + 8, :],
                            in_=wq_r[:, hh:hh + 8,
                                     col0 + cc0 * 128:col0 + (cc0 + 1) * 128])
                    return wt

                def qk_proj(col0, emit, dma_eng, preloaded=None, after_c=None):
                    """Project+rope cols [col0, col0+2048) of wqkv.

                    emit(c, rb, apA, apB): receive bf16 [128,512] rope outputs
                    for col-chunk c (head0) and c+8 (head1), row block rb."""
                    for c in range(8):
                        if c == 0 and preloaded is not None:
                            w1, w2 = preloaded
                        else:
                            w1 = load_wst(col0, c, dma_eng)
                            w2 = load_wst(col0, c + 8, dma_eng)
                        for rb in range(NRB):
                            rs = slice(rb * 512, (rb + 1) * 512)
                            ps1 = psp.tile([128, 512], F32, tag="mm")
                            ps2 = psp.tile([128, 512], F32, tag="mm")
                            for kc in range(16):
                                nc.tensor.matmul(ps1, w1[:, kc, :], x_sb[:, kc, rs],
                                                 start=kc == 0, stop=kc == 15)
                            for kc in range(16):
                                nc.tensor.matmul(ps2, w2[:, kc, :], x_sb[:, kc, rs],
                                                 start=kc == 0, stop=kc == 15)
                            cosv = cos_sb[:, c % 4, rs]
                            sinv = sin_sb[:, c % 4, rs]
                            t1 = projs.tile([128, 512], F32, tag="rt", bufs=4)
                            t2 = projs.tile([128, 512], F32, tag="rt", bufs=4)
                            outA = projs.tile([128, 512], BF, tag="ro", bufs=4)
                            outB = projs.tile([128, 512], BF, tag="ro", bufs=4)
                            nc.vector.tensor_mul(t1, ps1, cosv)
                            nc.vector.tensor_mul(t2, ps2, sinv)
                            nc.vector.tensor_sub(outA, t1, t2)
                            nc.vector.tensor_mul(t1, ps2, cosv)
                            nc.vector.tensor_mul(t2, ps1, sinv)
                            nc.vector.tensor_add(outB, t1, t2)
                            emit(c, rb, outA, outB)
                        if after_c is not None:
                            after_c(c)

                # ---- k projection + rope -> paired-row k_in, chunked AG
                def emit_k(c, rb, apA, apB):
                    rs = slice(rb * 512, (rb + 1) * 512)
                    nc.gpsimd.dma_start(
                        out=k_in[(2 * c) * 128:(2 * c + 1) * 128, rs], in_=apA)
                    nc.gpsimd.dma_start(
                        out=k_in[(2 * c + 1) * 128:(2 * c + 2) * 128, rs], in_=apB)

                def k_after_c(c):
                    if c == 3:
                        k_ag_chunk(0)
                    elif c == 7:
                        k_ag_chunk(1)

                qk_proj(D, emit_k, nc.scalar, preloaded=w_first,
                        after_c=k_after_c)

                # ---- q projection + rope -> qT_sb (resident)
                def emit_q(c, rb, apA, apB):
                    rs = slice(rb * 512, (rb + 1) * 512)
                    nc.vector.tensor_copy(qT_sb[:, c, rs], apA)
                    nc.vector.tensor_copy(qT_sb[:, c + 8, rs], apB)

                qk_proj(0, emit_q, nc.sync)

                # ---- v projection (block-column layout), chunked AG
                for vc in range(4):
                    if vc == 0:
                        wv = wv0
                    else:
                        wv = projs.tile([128, 16, 512], BF, tag="wv", bufs=2)
                        for kc in range(0, 16, 4):
                            nc.sync.dma_start(
                                out=wv[:, kc:kc + 4, :],
                                in_=wq_r[:, kc:kc + 4, 2 * D + vc * 512:2 * D + (vc + 1) * 512])
                    for rr in range(R // 128):
                        ps = psp.tile([128, 512], F32, tag="mm")
                        for kc in range(16):
                            nc.tensor.matmul(ps, x_sb[:, kc, rr * 128:(rr + 1) * 128],
                                             wv[:, kc, :], start=kc == 0, stop=kc == 15)
                        vt = projs.tile([128, 512], BF, tag="vo", bufs=4)
                        nc.vector.tensor_copy(vt, ps)
                        nc.gpsimd.dma_start(
                            out=v_in[vc * R + rr * 128:vc * R + (rr + 1) * 128, :],
                            in_=vt)
                    v_ag_chunk(vc)

            # ===================================================== attention
            with tc.tile_pool(name="attn", bufs=1) as attn:
                for hi in range(H):
                    kT_sb = attn.tile([128, 8, S], BF, tag="kT")
                    for sh in range(2):
                        for dc in range(8):
                            # paired row (2*dc+hi)*128 of the k shard lives in
                            # chunk h at k_g[h*2048 + sh*1024 + (row % 1024)]
                            row = (2 * dc + hi) * 128
                            base = (row // 1024) * 2048 + sh * 1024 + row % 1024
                            nc.scalar.dma_start(
                                out=kT_sb[:, dc, sh * R:(sh + 1) * R],
                                in_=k_g[base:base + 128, :])
                    v_sb = attn.tile([128, 16, HD], BF, tag="vh")
                    for bi in range(2):
                        for sh in range(2):
                            # col block b = 2*hi+bi of shard sh sits in the
                            # per-vc gather chunk b at v_g[b*2R + sh*R]
                            base = (2 * hi + bi) * 2 * R + sh * R
                            nc.sync.dma_start(
                                out=v_sb[:, sh * 8:(sh + 1) * 8,
                                         bi * 512:(bi + 1) * 512],
                                in_=v_g[base:base + R, :].rearrange(
                                    "(c p) m -> p c m", p=128))
                    for rb in range(NRB):
                        rs = slice(rb * 512, (rb + 1) * 512)
                        PT = attn.tile([128, 16, 512], BF, tag="PT", bufs=2)
                        for jc in range(16):
                            ps = psp.tile([128, 512], F32, tag="mm")
                            for dc in range(8):
                                nc.tensor.matmul(
                                    ps, kT_sb[:, dc, jc * 128:(jc + 1) * 128],
                                    qT_sb[:, hi * 8 + dc, rs],
                                    start=dc == 0, stop=dc == 7)
                            nc.scalar.activation(PT[:, jc, :], ps, Exp, scale=SCALE)
                        # row sums via ones-matmul, emitted AFTER two av
                        # chains so the PT exps are long done; the f32r
                        # broadcast matmul follows one more av chain so the
                        # scalar copy of the sums has landed.  The wide
                        # reciprocal runs on DVE off the PE critical path.
                        rec_b = attn.tile([128, 512], F32, tag="rec_b", bufs=2)
                        sums_sb = attn.tile([1, 512], F32R, tag="ssum", bufs=2)
                        pending = []
                        for m in range(8):
                            pa = psp.tile([128, 512], F32, tag="mm")
                            for jc in range(16):
                                nc.tensor.matmul(
                                    pa, v_sb[:, jc, m * 128:(m + 1) * 128],
                                    PT[:, jc, :], start=jc == 0, stop=jc == 15)
                            if m == 1:
                                sps = psp.tile([1, 512], F32, tag="sum", bufs=1)
                                for jc in range(16):
                                    nc.tensor.matmul(sps, ones_sb, PT[:, jc, :],
                                                     start=jc == 0, stop=jc == 15)
                                with nc.allow_low_precision(
                                        reason="f32r rowsums, f22 read by PE"):
                                    nc.scalar.copy(sums_sb, sps)
                            if m == 2:
                                rec_ps = psp.tile([128, 512], F32, tag="rb",
                                                  bufs=1)
                                nc.tensor.matmul(rec_ps,
                                                 onesr_sb.bitcast(F32R), sums_sb)
                                nc.vector.reciprocal(rec_b, rec_ps)
                            if m < 2:
                                # rec_b is written at m == 2; muls for the
                                # first chains are deferred until after it
                                pending.append((m, pa))
                                continue
                            for pm, ppa in pending:
                                nc.vector.tensor_mul(
                                    aoT_sb[:, hi * 8 + pm, rs], ppa, rec_b)
                            pending = []
                            nc.vector.tensor_mul(aoT_sb[:, hi * 8 + m, rs], pa, rec_b)

            # ============================================== output projection
            with tc.tile_pool(name="fin", bufs=1) as fin:
                wout_r = wout.rearrange("(c p) m -> p c m", p=128)
                for cc in range(4):
                    wo = fin.tile([128, 16, 512], BF, tag="wo", bufs=3)
                    for dc in range(0, 16, 2):
                        nc.scalar.dma_start(
                            out=wo[:, dc:dc + 2, :],
                            in_=wout_r[:, dc:dc + 2, cc * 512:(cc + 1) * 512])
                    for rr in range(R // 128):
                        r0 = rr * 128
                        ps = psp.tile([128, 512], F32, tag="mm")
                        for dc in range(16):
                            nc.tensor.matmul(ps, aoT_sb[:, dc, r0:r0 + 128],
                                             wo[:, dc, :],
                                             start=dc == 0, stop=dc == 15)
                        ot = fin.tile([128, 512], BF, tag="ot", bufs=4)
                        nc.vector.tensor_add(ot, ps, bias_sb[:, cc * 512:(cc + 1) * 512])
                        st_eng = nc.gpsimd if rr % 2 == 0 else nc.scalar
                        st_eng.dma_start(
                            out=out[r0:r0 + 128, cc * 512:(cc + 1) * 512], in_=ot)

    nc.compile()
    return nc


def _get_nc():
    if "nc" not in _NC_CACHE:
        _NC_CACHE["nc"] = _build()
    return _NC_CACHE["nc"]


def _rope_tables():
    inv_freq = 1.0 / (10000.0 ** (np.arange(0, HD, 2, dtype=np.float32) / HD))
    t = np.arange(S, dtype=np.float32)
    freqs = t[:, None] * inv_freq[None, :]          # (S, 512)
    return np.cos(freqs).astype(np.float32), np.sin(freqs).astype(np.float32)


def kernel(x, wqkv, wout, bout):
    from concourse.bass_utils import run_bass_kernel_spmd

    bf16 = ml_dtypes.bfloat16
    x = np.asarray(x, dtype=np.float32)
    wqkv_b = np.ascontiguousarray(np.asarray(wqkv, dtype=np.float32)).astype(bf16)
    wout_b = np.ascontiguousarray(np.asarray(wout, dtype=np.float32)).astype(bf16)
    bout_f = np.asarray(bout, dtype=np.float32).reshape(1, D)
    cos_h, sin_h = _rope_tables()                   # (S, 512) f32
    cosT = np.ascontiguousarray(cos_h.T)            # (512, S)
    sinT = np.ascontiguousarray(sin_h.T)

    nc = _get_nc()

    in_maps = []
    for c in range(N_CORES):
        bi, half = c // 2, c % 2
        rows = slice(half * R, (half + 1) * R)
        in_maps.append({
            "wqkv": wqkv_b,
            "wout": wout_b,
            "bias": bout_f,
            "xT": np.ascontiguousarray(x[bi, rows, :].T).astype(bf16),
            "cost": np.ascontiguousarray(cosT[:, rows]).astype(np.float16),
            "sint": np.ascontiguousarray(sinT[:, rows]).astype(np.float16),
        })

    trace = os.environ.get("KERNEL_TRACE", "0") == "1"
    res = run_bass_kernel_spmd(nc, in_maps, list(range(N_CORES)), trace=trace)
    if trace:
        LAST_RESULT["exec_time_ns"] = res.exec_time_ns
        LAST_RESULT["mean_exec_time_ns"] = res.mean_exec_time_ns

    out_full = np.empty((B, S, D), np.float32)
    for c in range(N_CORES):
        bi, half = c // 2, c % 2
        out_full[bi, half * R:(half + 1) * R, :] = \
            np.asarray(res.results[c]["out"]).astype(np.float32)
    return out_full
